# revision 13
# baseline (speedup 1.0000x reference)
"""Longformer sliding-chunk attention (B=2, S=4096, E=1024, H=16, W=256) on 8 trn2 cores.

Sharding: tensor-parallel over heads — core c owns heads {2c, 2c+1}. Each core:
  - projects q/k/v for its 128 output features (2 heads x 64) over the full
    [8192, 1024] hidden states, in transposed [d, s] layout, bf16
  - computes chunked attention fully transposed and software-pipelined with the
    projections (per 512-seq group: project, transpose new V blocks, compute
    scoresT = K @ Q^T per 128-key-block over its full 768-query window, exp on
    ACT into a bf16 probs ring, then probsT @ V for completed chunks with an
    appended ones-column yielding softmax denominators for free)
  - ships unnormalized numerator^T [128, 8192] + denominators [2, 8192]
Host adds the boundary-mask pad mass to denominators and normalizes.

All matmuls run in bf16 (fp32 PSUM accumulation). No max-subtraction before
exp: scores are O(1) for this problem.
"""
import numpy as np
import ml_dtypes

import concourse.bass as bass
import concourse.mybir as mybir
import concourse.tile as tile
from concourse import bacc
from concourse.bass_utils import run_bass_kernel_spmd
from concourse.masks import make_identity

F32 = mybir.dt.float32
BF16 = mybir.dt.bfloat16
AFT = mybir.ActivationFunctionType

B, S, E = 2, 4096, 1024
H, W, D = 16, 256, 64
BS = B * S           # 8192
KT = 8               # contraction tiles of 128 over E
NCHUNK = S // W      # 16 chunks per batch
NKB = S // 128       # 32 key blocks of 128 per batch
NG = 8               # 512-seq groups per batch
PR = 10              # probs ring slots per head
VR = 16              # vring slots

_NC_CACHE = None


def _score_window(kb):
    """Chunk range [w0, w1) of queries attending key block kb."""
    w0 = max(0, kb // 2 - 1)
    w1 = min(NCHUNK, kb // 2 + 2)
    return w0, w1


def _scores_ready(kb, g):
    """Can scores for key block kb be emitted after local group g of its batch?"""
    have = (g + 1) * 512
    if (kb + 1) * 128 > have:
        return False
    _, w1 = _score_window(kb)
    return w1 * 256 <= have


def _build():
    nc = bacc.Bacc("TRN2", target_bir_lowering=False, debug=False, num_devices=8)

    # host pre-arranges inputs partition-major so every DMA is contiguous
    # per partition: hsT [p, group, ktile, 512], wAll [p, proj, ktile, 128]
    hsT = nc.dram_tensor("hsT", [128, B * NG * KT * 512], BF16,
                         kind="ExternalInput").ap()
    wAll = nc.dram_tensor("wAll", [128, 3 * KT * 128], BF16,
                          kind="ExternalInput").ap()
    bAll = nc.dram_tensor("bAll", [128, 3], F32, kind="ExternalInput").ap()
    onesd = nc.dram_tensor("onesd", [128, 2 * VR], BF16, kind="ExternalInput").ap()
    outT = nc.dram_tensor("outT", [130, BS], F32, kind="ExternalOutput").ap()

    with tile.TileContext(nc) as tc:
        with (
            tc.tile_pool(name="singles", bufs=1) as singles,
            tc.tile_pool(name="big", bufs=1) as big,
            tc.tile_pool(name="hst", bufs=3) as hpool,
            tc.tile_pool(name="stage", bufs=4) as stage_pool,
            tc.tile_pool(name="den", bufs=4) as den_pool,
            tc.tile_pool(name="psB", bufs=2, space="PSUM") as psB,   # proj + scores
            tc.tile_pool(name="psC", bufs=3, space="PSUM") as psC,   # pv accumulators
            tc.tile_pool(name="psD", bufs=1, space="PSUM") as psD,   # v transposes
        ):
            hsT_r = hsT.rearrange("p (g kt s) -> p g kt s", g=B * NG, kt=KT)

            # weights/biases first (single packed DMAs on SyncE), first input
            # group split per k-tile on GpSimd so the first matmul can start
            # as early as possible
            w_all = singles.tile([128, 3, KT, 128], BF16, tag="wall")
            nc.sync.dma_start(
                out=w_all, in_=wAll.rearrange("p (w kt m) -> p w kt m", w=3, kt=KT)
            )
            b_all = singles.tile([128, 3], F32, tag="ball")
            nc.sync.dma_start(out=b_all, in_=bAll)
            w_sb = {nm: w_all[:, i] for i, nm in enumerate(("q", "k", "v"))}
            b_sb = {nm: b_all[:, i : i + 1] for i, nm in enumerate(("q", "k", "v"))}

            gt0 = hpool.tile([128, KT, 512], BF16, tag="hst", name="hst_g0")
            for kt in range(KT):
                nc.gpsimd.dma_start(out=gt0[:, kt, :], in_=hsT_r[:, 0, kt, :])

            QT = big.tile([128, BS], BF16, tag="qt")
            KTt = big.tile([128, BS], BF16, tag="kt")
            VT = big.tile([128, BS], BF16, tag="vt")
            vring = big.tile([128, VR, 130], BF16, tag="vring")
            nc.sync.dma_start(
                out=vring.rearrange("p s (x o) -> p s x o", x=2)[:, :, :, 64:65],
                in_=onesd.rearrange("p (s x o) -> p s x o", s=VR, x=2, o=1),
            )
            probs = {
                h: big.tile([128, PR, 768], BF16, tag=f"probs{h}", name=f"probs{h}")
                for h in (0, 1)
            }

            ident = singles.tile([128, 128], BF16)
            make_identity(nc, ident)

            def emit_transpose(b, kb):
                base = b * S
                vt = psD.tile([128, 128], BF16, tag="vt")
                nc.tensor.transpose(
                    vt, VT[:, base + kb * 128 : base + (kb + 1) * 128], ident
                )
                slot = (b * NKB + kb) % VR
                nc.vector.tensor_copy(
                    vring[:, slot, :].rearrange("p (h x) -> p h x", h=2)[:, :, 0:64],
                    vt.rearrange("p (h x) -> p h x", h=2),
                )

            def emit_scores(b, kb, h):
                base = b * S
                w0, w1 = _score_window(kb)
                q0 = base + w0 * 256
                width = (w1 - w0) * 256
                d_sl = slice(h * 64, (h + 1) * 64)
                k_sl = slice(base + kb * 128, base + (kb + 1) * 128)
                sp = psB.tile([128, 1024], F32, tag="mm")
                nc.tensor.matmul(
                    sp[:, 0:512],
                    lhsT=KTt[d_sl, k_sl],
                    rhs=QT[d_sl, q0 : q0 + 512],
                    start=True, stop=True,
                )
                if width > 512:
                    nc.tensor.matmul(
                        sp[:, 512:768],
                        lhsT=KTt[d_sl, k_sl],
                        rhs=QT[d_sl, q0 + 512 : q0 + 768],
                        start=True, stop=True,
                    )
                slot = (b * NKB + kb) % PR
                nc.scalar.activation(
                    probs[h][:, slot, 0:width], sp[:, 0:width], AFT.Exp
                )

            def emit_chunk(b, c):
                base = b * S
                lo = max(0, 2 * c - 2)
                hi = min(NKB, 2 * c + 4)
                o_sl = slice(base + c * W, base + (c + 1) * W)
                stage = stage_pool.tile([128, 256], F32, tag="stage")
                for h in (0, 1):
                    po = psC.tile([65, 256], F32, tag="pv")
                    for i, kb in enumerate(range(lo, hi)):
                        w0, _ = _score_window(kb)
                        slot = (b * NKB + kb) % PR
                        off = (c - w0) * 256
                        nc.tensor.matmul(
                            po,
                            lhsT=vring[
                                :, (b * NKB + kb) % VR, h * 65 : (h + 1) * 65
                            ],
                            rhs=probs[h][:, slot, off : off + 256],
                            start=(i == 0),
                            stop=(i == hi - lo - 1),
                        )
                    nc.vector.tensor_copy(
                        stage[h * 64 : (h + 1) * 64, :], po[0:64, :]
                    )
                    den_h = den_pool.tile(
                        [1, 256], F32, tag=f"den{h}", name=f"den{h}_{b}_{c}"
                    )
                    nc.vector.tensor_copy(den_h, po[64:65, :])
                    nc.sync.dma_start(out=outT[128 + h : 129 + h, o_sl], in_=den_h)
                nc.sync.dma_start(out=outT[0:128, o_sl], in_=stage)

            # Software pipeline: phase-2 work that becomes ready after local
            # group lg is emitted one global group later, so the PE never
            # waits on the just-written Q/K/V of the current group.
            state = [{"kb": 0, "c": 0} for _ in range(B)]
            for gg in range(B * NG + 1):
                if gg < B * NG:
                    b, g = divmod(gg, NG)
                    gsl = slice(gg * 512, (gg + 1) * 512)
                    if gg == 0:
                        gt = gt0
                    else:
                        gt = hpool.tile([128, KT, 512], BF16, tag="hst")
                        nc.gpsimd.dma_start(out=gt, in_=hsT_r[:, gg])
                    for nm, dest, scale in (
                        ("q", QT, 1.0 / np.sqrt(D)),
                        ("k", KTt, 1.0),
                        ("v", VT, 1.0),
                    ):
                        ps = psB.tile([128, 1024], F32, tag="mm")
                        for kt in range(KT):
                            nc.tensor.matmul(
                                ps[:, 0:512],
                                lhsT=w_sb[nm][:, kt, :],
                                rhs=gt[:, kt, :],
                                start=(kt == 0),
                                stop=(kt == KT - 1),
                            )
                        nc.scalar.activation(
                            dest[:, gsl], ps[:, 0:512], AFT.Identity,
                            bias=b_sb[nm], scale=scale,
                        )

                pg = gg - 1
                if pg < 0:
                    continue
                b2, lg = divmod(pg, NG)
                st = state[b2]
                t_kbs = list(range(4 * lg, min(4 * lg + 4, NKB)))
                s_kbs = []
                while st["kb"] < NKB and _scores_ready(st["kb"], lg):
                    s_kbs.append(st["kb"])
                    st["kb"] += 1
                c_done = []
                while st["c"] < NCHUNK and min(NKB, 2 * st["c"] + 4) <= st["kb"]:
                    c_done.append(st["c"])
                    st["c"] += 1

                ti = 0
                for kb in s_kbs:
                    if ti < len(t_kbs):
                        emit_transpose(b2, t_kbs[ti])
                        ti += 1
                    for h in (0, 1):
                        emit_scores(b2, kb, h)
                while ti < len(t_kbs):
                    emit_transpose(b2, t_kbs[ti])
                    ti += 1
                for c in c_done:
                    emit_chunk(b2, c)

    nc.compile()
    return nc


def get_nc():
    global _NC_CACHE
    if _NC_CACHE is None:
        _NC_CACHE = _build()
    return _NC_CACHE


def make_in_maps(hidden_states, Wq, bq, Wk, bk, Wv, bv):
    bf16 = ml_dtypes.bfloat16
    # hsT partition-major: [p, group, ktile, 512] flattened to [128, 65536]
    hsT = (
        hidden_states.reshape(BS, E)
        .T.astype(bf16)                       # [E, BS] = [kt*128+p, g*512+x]
        .reshape(KT, 128, B * NG, 512)
        .transpose(1, 2, 0, 3)
        .reshape(128, B * NG * KT * 512)
    )
    onesd = np.ones((128, 2 * VR), bf16)
    in_maps = []
    for c in range(8):
        fsl = slice(c * 128, (c + 1) * 128)
        # wAll partition-major: [p, proj, ktile, 128] flattened to [128, 3072]
        wAll = (
            np.stack(
                [
                    Wm[fsl].T.astype(np.float32).reshape(KT, 128, 128)
                    for Wm in (Wq, Wk, Wv)
                ],
                axis=0,
            )                                  # [w, kt, p, m]
            .transpose(2, 0, 1, 3)
            .reshape(128, 3 * KT * 128)
            .astype(bf16)
        )
        bAll = np.stack(
            [
                bq[fsl].astype(np.float32) / np.sqrt(D),
                bk[fsl].astype(np.float32),
                bv[fsl].astype(np.float32),
            ],
            axis=1,
        )
        in_maps.append(
            {
                "hsT": np.ascontiguousarray(hsT),
                "wAll": np.ascontiguousarray(wAll),
                "bAll": np.ascontiguousarray(bAll),
                "onesd": onesd,
            }
        )
    return in_maps


def assemble(results):
    """results: list of 8 per-core dicts with 'outT' [130, BS] -> full [B,S,E]."""
    # boundary pad mass: chunk 0 row ii has ii unmasked zero-score pad keys,
    # chunk 15 row ii has 255-ii
    pad = np.zeros(S, np.float32)
    pad[:W] = np.arange(W, dtype=np.float32)
    pad[S - W :] = (W - 1) - np.arange(W, dtype=np.float32)

    out = np.empty((B, S, E), np.float32)
    for c in range(8):
        oT = results[c]["outT"]  # [130, BS]
        num = oT[0:128].T.reshape(B, S, 2, 64)  # b, s, head_local, d
        den = oT[128:130].T.reshape(B, S, 2)  # b, s, head_local
        den = den + pad[None, :, None]
        out[:, :, c * 128 : (c + 1) * 128] = (num / den[..., None]).reshape(B, S, 128)
    return out


def kernel(hidden_states, Wq, bq, Wk, bk, Wv, bv):
    nc = get_nc()
    in_maps = make_in_maps(hidden_states, Wq, bq, Wk, bk, Wv, bv)
    res = run_bass_kernel_spmd(nc, in_maps, list(range(8)))
    return assemble(res.results)
